# revision 110
# baseline (speedup 1.0000x reference)
"""Trainium2 Bass kernel for BatchShawMultigraphAttention.

Math (derived from the reference):
  - attn_biases adds a per-row constant to scores -> cancels in softmax.
  - w.sum(-1) == 1 after softmax, so the bias term reduces to "+ biases[e,h]".
  - masked softmax with -1e10 == multiply exp(scores) by binary A (rows are
    never fully masked at 10% density, N=1024).
  So per (b,e,h):
    P = exp(q @ k^T / sqrt(F_));  T = A * P
    out = relu( (T @ (v + bias_eh)) / (T @ 1) )

Sharding: 8 cores = (b in 0..3) x (query-row half in 0..1); each core owns
512 softmax rows for all (e,h), reading its A slice exactly once.

Device schedule (everything transposed [j, i]; fp16 compute, fp32 PSUM,
fp16 output staging):
  - P^T tiles per h from fp16 q/k matmuls + one exp per [128,1024] chunk.
  - T^T = A^T * P^T on the DVE (2x fp16 mode) for most (h,e) pairs, in
    [128,1024] chunks early (to track the exp pipeline) and whole
    [128,4096] tiles later, ordered by operand readiness.
  - The FUSED (h,e) pairs bypass the DVE: the mask is accumulated into the
    score PSUM by a diag(30000) matmul against A^T and folded through the
    exp bias (exp(s + 30000 A - 30000) = A * exp(s) for binary A), letting
    the otherwise-idle Activation engine absorb tail mask work.
  - Row sums ride along as a ones column of va; softmax normalization is a
    batched DVE reciprocal per (e,h) group followed by either a DVE
    tensor_scalar(mult, max) or, for groups landing after the exp chain
    drains, an Act Relu-with-scale (only hardware-proven primitives: the
    Pool engine cannot touch PSUM and the divide ALU op fails ISA checks).
"""

import sys

sys.path.insert(0, "/opt/trn_rl_repo")

import numpy as np
import ml_dtypes

B, E, H, N, F, F_ = 4, 4, 4, 1024, 64, 32
NCORES = 8
IH = N // 2          # 512 query rows per core
JB = N // 128        # 8 key blocks
VA_W = F_ + 1        # v columns + ones column = 33
EH = E * H
MASK_BIG = 30000.0

# (h, e) pairs whose masking runs on the Activation engine (late tiles);
# everything else is a DVE multiply.
# (h, e) pairs whose masking runs via the fused PE-mask + Act-exp path.
FUSED = [(3, 1), (3, 2), (3, 3)]
# DVE full-tile order after the chunked (h<2, e<2) ramp, by readiness.
FULL_ORDER = [(0, 2), (1, 2), (0, 3), (3, 0), (2, 0), (1, 3), (2, 1),
              (2, 2), (2, 3)]
WARMUP = 90
TT_BUFS = 10
EMISSION = None     # optional override of the phase C emission schedule

_compiled = None


def _build():
    import concourse.bass as bass
    import concourse.bacc as bacc
    import concourse.tile as tile
    import concourse.mybir as mybir

    f32 = mybir.dt.float32
    f16 = mybir.dt.float16
    bf16 = mybir.dt.bfloat16
    nc = bacc.Bacc("TRN2", target_bir_lowering=False, debug=False,
                   enable_asserts=False, num_devices=NCORES)

    qt_d = nc.dram_tensor("qt", [F_, H * IH], f16, kind="ExternalInput")
    kt_d = nc.dram_tensor("kt", [F_, H * N], f16, kind="ExternalInput")
    va_d = nc.dram_tensor("va", [128, EH * JB * VA_W], f16,
                          kind="ExternalInput")
    at_d = nc.dram_tensor("at", [E, 128, JB * IH], f16, kind="ExternalInput")
    dg_d = nc.dram_tensor("dg", [128, 128], f16, kind="ExternalInput")
    out_d = nc.dram_tensor("out", [IH // 128, 128, EH * F_], f16,
                           kind="ExternalOutput")

    scale = float(1.0 / np.sqrt(F_))
    div_max = (mybir.AluOpType.divide, mybir.AluOpType.max)
    Exp = mybir.ActivationFunctionType.Exp

    with tile.TileContext(nc) as tc:
        with (
            tc.tile_pool(name="const", bufs=1) as cpool,
            tc.tile_pool(name="pt", bufs=1) as ptpool,
            tc.tile_pool(name="tt", bufs=TT_BUFS) as ttpool,
            tc.tile_pool(name="st", bufs=2, space=bass.MemorySpace.PSUM) as stpool,
            tc.tile_pool(name="po", bufs=4, space=bass.MemorySpace.PSUM) as popool,
        ):
            # PE p-state warmup: harmless matmuls on a memset tile keep the
            # tensor engine busy through the initial DMA wait so real work
            # starts at full clock.
            if WARMUP:
                wsrc = cpool.tile([F_, 256], f16, tag="wsrc")
                nc.gpsimd.memset(wsrc[:], 0.0)
                wps = popool.tile([128, IH // 128, VA_W], f32, tag="po",
                                  name="wps")
                for i in range(WARMUP):
                    nc.tensor.matmul(wps[:, 0, :], wsrc[:, 0:128],
                                     wsrc[:, 0:VA_W], start=True,
                                     stop=True)

            # --- input DMAs, ordered for earliest compute start ---
            kt = cpool.tile([F_, H * N], f16, tag="kt")
            qt = cpool.tile([F_, H * IH], f16, tag="qt")
            at_t = []
            for e in range(E):
                at = cpool.tile([128, JB * IH], f16, tag=f"at{e}")
                at_t.append(at)
            Q = 2 * IH                                           # quarter
            nc.sync.dma_start(kt[:, 0:N], kt_d[:, 0:N])          # h0 first
            nc.sync.dma_start(qt[:, 0:IH], qt_d[:, 0:IH])
            dg = cpool.tile([128, 128], f16, tag="dg")
            nc.sync.dma_start(dg[:], dg_d[:])
            for q in range(2):                                   # quarters 0,1
                for e in range(2):
                    nc.sync.dma_start(at_t[e][:, q * Q:(q + 1) * Q],
                                      at_d[e, :, q * Q:(q + 1) * Q])
            nc.sync.dma_start(kt[:, N:2 * N], kt_d[:, N:2 * N])  # h1
            nc.sync.dma_start(qt[:, IH:2 * IH], qt_d[:, IH:2 * IH])
            for q in range(2, 4):                                # quarters 2,3
                for e in range(2):
                    nc.sync.dma_start(at_t[e][:, q * Q:(q + 1) * Q],
                                      at_d[e, :, q * Q:(q + 1) * Q])
            nc.sync.dma_start(kt[:, 2 * N:], kt_d[:, 2 * N:])    # h2, h3
            nc.sync.dma_start(qt[:, 2 * IH:], qt_d[:, 2 * IH:])
            nc.sync.dma_start(at_t[2][:, 0:4 * IH], at_d[2, :, 0:4 * IH])
            nc.sync.dma_start(at_t[2][:, 4 * IH:], at_d[2, :, 4 * IH:])
            nc.sync.dma_start(at_t[3][:], at_d[3])
            va = cpool.tile([128, EH * JB * VA_W], f16, tag="va")
            nc.sync.dma_start(va[:], va_d[:])
            mbias = cpool.tile([128, 1], f32, tag="mbias")
            nc.gpsimd.memset(mbias[:], float(-MASK_BIG * scale))

            # --- phase P: pt[h][j-part, jb*IH + i] = exp(k_j . q_i * scale)
            pt_t = []
            for h in range(H):
                pt = ptpool.tile([128, JB * IH], f16, tag=f"pt{h}",
                                 name=f"pt{h}")
                pt_t.append(pt)

            def qk_pair(h, jbp, st, start=True, stop=True):
                for half in range(2):
                    jb = 2 * jbp + half
                    nc.tensor.matmul(
                        st[:, half * IH:(half + 1) * IH],
                        kt[:, h * N + jb * 128:h * N + (jb + 1) * 128],
                        qt[:, h * IH:(h + 1) * IH],
                        start=start, stop=stop)

            # --- phase C ---
            half = EH * F_ // 2
            stage = cpool.tile([128, IH // 128, EH * F_], f16, tag="stage")

            nb = IH // 128
            po_map = {}

            def po_group(h, e, tt):
                vab = (e * H + h) * JB * VA_W
                po = popool.tile([128, nb, VA_W], f32, tag="po")
                for ib in range(nb):
                    for jb in range(JB):
                        nc.tensor.matmul(
                            po[:, ib, :],
                            tt[:, jb * IH + ib * 128:jb * IH + ib * 128 + 128],
                            va[:, vab + jb * VA_W:vab + (jb + 1) * VA_W],
                            start=(jb == 0), stop=(jb == JB - 1))
                po_map[h, e] = po

            Relu = mybir.ActivationFunctionType.Relu

            def div_group(h, e, on_act=False):
                # divide+ReLU via per-partition scalar pointer on the DVE
                # (scalar operands are exempt from the one-PSUM-input rule);
                # late groups run as recip+relu on the Act engine, which has
                # drained its exp queue by then.
                col = (e * H + h) * F_
                po = po_map[h, e]
                rec = cpool.tile([128, nb, 1], f32, tag="rec", bufs=16)
                nc.vector.reciprocal(rec[:], po[:, :, F_:F_ + 1])
                for ib in range(nb):
                    if on_act:
                        nc.scalar.activation(
                            stage[:, ib, col:col + F_], po[:, ib, 0:F_],
                            Relu, scale=rec[:, ib, :])
                    else:
                        nc.vector.tensor_scalar(
                            stage[:, ib, col:col + F_], po[:, ib, 0:F_],
                            rec[:, ib, :], 0.0, mybir.AluOpType.mult,
                            mybir.AluOpType.max)

            tt_map = {}

            def fused_chunk(h, e, jbp):
                if (h, e) not in tt_map:
                    tt_map[h, e] = ttpool.tile([128, JB * IH], f16, tag="tt",
                                               name=f"ttf{h}{e}")
                tt = tt_map[h, e]
                st = stpool.tile([128, 2 * IH], f32, tag="st", name="stf")
                for half in range(2):
                    jb = 2 * jbp + half
                    sl = st[:, half * IH:(half + 1) * IH]
                    nc.tensor.matmul(
                        sl, kt[:, h * N + jb * 128:h * N + (jb + 1) * 128],
                        qt[:, h * IH:(h + 1) * IH], start=True, stop=False)
                    nc.tensor.matmul(
                        sl, dg[:], at_t[e][:, jb * IH:(jb + 1) * IH],
                        start=False, stop=True)
                nc.scalar.activation(
                    tt[:, jbp * 2 * IH:(jbp + 1) * 2 * IH], st[:], Exp,
                    scale=scale, bias=mbias[:])

            # phase P exps
            for h in range(H):
                for jbp in range(JB // 2):
                    st = stpool.tile([128, 2 * IH], f32, tag="st", name="stm")
                    qk_pair(h, jbp, st)
                    nc.scalar.activation(
                        pt_t[h][:, jbp * 2 * IH:(jbp + 1) * 2 * IH], st[:],
                        Exp, scale=scale)

            # DVE stream: chunked (h<2, e<2) ramp tracking the exp pipeline,
            # then whole-tile multiplies in readiness order.
            for h in range(2):
                for e in range(2):
                    tt_map[h, e] = ttpool.tile([128, JB * IH], f16, tag="tt",
                                               name=f"tt{h}{e}")
            for h in range(2):
                for jbp in range(JB // 2):
                    for e in range(2):
                        c0 = jbp * 2 * IH
                        nc.vector.tensor_mul(
                            tt_map[h, e][:, c0:c0 + 2 * IH],
                            at_t[e][:, c0:c0 + 2 * IH],
                            pt_t[h][:, c0:c0 + 2 * IH])

            # PE/Act/Pool stream: remaining fused chunks and po groups in
            # predicted-runnability order; output DMAs for each column half
            # right after that half's last contributing divide.
            f1, f2, f3 = FUSED
            emission = EMISSION or [
                ("po", 0, 0), ("po", 0, 1), ("po", 1, 0), ("po", 1, 1),
                ("f", *f2, 0), ("mul", 0, 2), ("po", 0, 2),
                ("f", *f2, 1), ("mul", 1, 2), ("po", 1, 2),
                ("f", *f2, 2), ("mul", 0, 3), ("po", 0, 3),
                ("f", *f2, 3), ("mul", 3, 0), ("po", 3, 0),
                ("div", 0, 0), ("div", 0, 1), ("div", 1, 0),
                ("mul", *f1), ("po", *f1),
                ("div", 1, 1), ("div", 0, 2), ("div", 1, 2),
                ("f", *f3, 0), ("mul", 2, 0), ("po", 2, 0),
                ("div", 0, 3),
                ("f", *f3, 1), ("mul", 1, 3), ("po", 1, 3), ("div", 3, 0),
                ("f", *f3, 2), ("mul", 2, 1), ("po", 2, 1), ("div", 2, 0),
                ("div", *f1),
                ("f", *f3, 3), ("mul", 2, 2), ("po", 2, 2), ("div", 1, 3),
                ("div", 2, 1),
                ("po", *f2), ("div", *f2),
                ("mul", 2, 3), ("po", 2, 3),
                ("po", *f3), ("div", 2, 2), ("div", *f3), ("div", 2, 3),
                ("out", 0), ("out", 1),
            ]
            ndiv = 0
            for act in emission:
                if act[0] == "f":
                    fused_chunk(act[1], act[2], act[3])
                elif act[0] == "mul":
                    h, e = act[1], act[2]
                    tt_map[h, e] = ttpool.tile([128, JB * IH], f16, tag="tt",
                                               name=f"ttm{h}{e}")
                    nc.vector.tensor_mul(tt_map[h, e][:], at_t[e][:],
                                         pt_t[h][:])
                elif act[0] == "po":
                    po_group(act[1], act[2], tt_map[act[1], act[2]])
                elif act[0] == "div":
                    late = act in (("div", 2, 0), ("div", 3, 0),
                                   ("div", 2, 1), ("div", 1, 3),
                                   ("div", *f1))
                    with tc.tile_wait_until((20.0 + 1.6 * ndiv) / 1000.0):
                        div_group(act[1], act[2], on_act=late)
                    ndiv += 1
                elif act[0] == "out":
                    for ib in (act[1], act[1] + 2):
                        nc.sync.dma_start(out_d[ib], stage[:, ib, :])

    nc.compile()
    return nc


def _prep_core_inputs(b, ih, X, A, kernel_w, biases, aks, akn):
    i0 = ih * IH
    Xb = X[b]                                        # [N, F]
    # qt[kk, h*IH + i] = sum_f X[i0+i, f] aks[h, f, kk]
    qt = np.einsum("nf,hfk->khn", Xb[i0:i0 + IH], aks).reshape(F_, H * IH)
    kt = np.einsum("nf,hfk->khn", Xb, akn).reshape(F_, H * N)
    v = np.einsum("nf,hfk->hnk", Xb, kernel_w)       # [H, N, F_]
    va = np.empty((E, H, JB, 128, VA_W), np.float32)
    for e in range(E):
        for h in range(H):
            vb = v[h] + biases[e, h][None, :]        # [N, F_]
            va[e, h, :, :, :F_] = vb.reshape(JB, 128, F_)
            va[e, h, :, :, F_] = 1.0
    # va2[p, ((e*H+h)*JB+jb)*33 + c]
    va2 = va.transpose(3, 0, 1, 2, 4).reshape(128, EH * JB * VA_W)
    # at[e, p, jb*IH + i] = A[b, e, i0+i, jb*128+p]
    at = (A[b, :, i0:i0 + IH, :]                     # [E, i, j]
          .transpose(0, 2, 1)                        # [E, j, i]
          .reshape(E, JB, 128, IH)
          .transpose(0, 2, 1, 3)                     # [E, p, jb, i]
          .reshape(E, 128, JB * IH))
    dg = np.diag(np.full(128, MASK_BIG, np.float32))
    return {"qt": qt.astype(np.float16),
            "kt": kt.astype(np.float16),
            "va": va2.astype(np.float16),
            "at": at.astype(np.float16),
            "dg": dg.astype(np.float16)}


def kernel(X, A, kernel, biases, attn_kernel_self, attn_kernel_neighs,
           attn_biases):
    global _compiled
    from concourse import bass_utils

    if _compiled is None:
        _compiled = _build()

    in_maps = [
        _prep_core_inputs(c // 2, c % 2, np.asarray(X), np.asarray(A),
                          np.asarray(kernel), np.asarray(biases),
                          np.asarray(attn_kernel_self),
                          np.asarray(attn_kernel_neighs))
        for c in range(NCORES)
    ]
    res = bass_utils.run_bass_kernel_spmd(_compiled, in_maps,
                                          core_ids=list(range(NCORES)))
    out = np.empty((B, N, EH * F_), np.float32)
    for c in range(NCORES):
        b, ih = c // 2, c % 2
        out[b, ih * IH:(ih + 1) * IH, :] = np.asarray(
            res.results[c]["out"]).reshape(IH, EH * F_).astype(np.float32)
    return out


# revision 113
# speedup vs baseline: 1.0005x; 1.0005x over previous
"""Trainium2 Bass kernel for BatchShawMultigraphAttention.

Math (derived from the reference):
  - attn_biases adds a per-row constant to scores -> cancels in softmax.
  - w.sum(-1) == 1 after softmax, so the bias term reduces to "+ biases[e,h]".
  - masked softmax with -1e10 == multiply exp(scores) by binary A (rows are
    never fully masked at 10% density, N=1024).
  So per (b,e,h):
    P = exp(q @ k^T / sqrt(F_));  T = A * P
    out = relu( (T @ (v + bias_eh)) / (T @ 1) )

Sharding: 8 cores = (b in 0..3) x (query-row half in 0..1); each core owns
512 softmax rows for all (e,h), reading its A slice exactly once.

Device schedule (everything transposed [j, i]; fp16 compute, fp32 PSUM,
fp16 output staging):
  - P^T tiles per h from fp16 q/k matmuls + one exp per [128,1024] chunk.
  - T^T = A^T * P^T on the DVE (2x fp16 mode) for most (h,e) pairs, in
    [128,1024] chunks early (to track the exp pipeline) and whole
    [128,4096] tiles later, ordered by operand readiness.
  - The FUSED (h,e) pairs bypass the DVE: the mask is accumulated into the
    score PSUM by a diag(30000) matmul against A^T and folded through the
    exp bias (exp(s + 30000 A - 30000) = A * exp(s) for binary A), letting
    the otherwise-idle Activation engine absorb tail mask work.
  - Row sums ride along as a ones column of va; softmax normalization is a
    batched DVE reciprocal per (e,h) group followed by either a DVE
    tensor_scalar(mult, max) or, for groups landing after the exp chain
    drains, an Act Relu-with-scale (only hardware-proven primitives: the
    Pool engine cannot touch PSUM and the divide ALU op fails ISA checks).
"""

import sys

sys.path.insert(0, "/opt/trn_rl_repo")

import numpy as np
import ml_dtypes

B, E, H, N, F, F_ = 4, 4, 4, 1024, 64, 32
NCORES = 8
IH = N // 2          # 512 query rows per core
JB = N // 128        # 8 key blocks
VA_W = F_ + 1        # v columns + ones column = 33
EH = E * H
MASK_BIG = 30000.0

# (h, e) pairs whose masking runs on the Activation engine (late tiles);
# everything else is a DVE multiply.
# (h, e) pairs whose masking runs via the fused PE-mask + Act-exp path.
FUSED = [(3, 1), (3, 2), (3, 3)]
# DVE full-tile order after the chunked (h<2, e<2) ramp, by readiness.
FULL_ORDER = [(0, 2), (1, 2), (0, 3), (3, 0), (2, 0), (1, 3), (2, 1),
              (2, 2), (2, 3)]
WARMUP = 90
TT_BUFS = 10
EMISSION = None     # optional override of the phase C emission schedule

_compiled = None


def _build():
    import concourse.bass as bass
    import concourse.bacc as bacc
    import concourse.tile as tile
    import concourse.mybir as mybir

    f32 = mybir.dt.float32
    f16 = mybir.dt.float16
    bf16 = mybir.dt.bfloat16
    nc = bacc.Bacc("TRN2", target_bir_lowering=False, debug=False,
                   enable_asserts=False, num_devices=NCORES)

    qt_d = nc.dram_tensor("qt", [F_, H * IH], f16, kind="ExternalInput")
    kt_d = nc.dram_tensor("kt", [F_, H * N], f16, kind="ExternalInput")
    va_d = nc.dram_tensor("va", [128, EH * JB * VA_W], f16,
                          kind="ExternalInput")
    at_d = nc.dram_tensor("at", [E, 128, JB * IH], f16, kind="ExternalInput")
    dg_d = nc.dram_tensor("dg", [128, 128], f16, kind="ExternalInput")
    out_d = nc.dram_tensor("out", [IH // 128, 128, EH * F_], f16,
                           kind="ExternalOutput")

    scale = float(1.0 / np.sqrt(F_))
    div_max = (mybir.AluOpType.divide, mybir.AluOpType.max)
    Exp = mybir.ActivationFunctionType.Exp

    with tile.TileContext(nc) as tc:
        with (
            tc.tile_pool(name="const", bufs=1) as cpool,
            tc.tile_pool(name="pt", bufs=1) as ptpool,
            tc.tile_pool(name="tt", bufs=TT_BUFS) as ttpool,
            tc.tile_pool(name="st", bufs=2, space=bass.MemorySpace.PSUM) as stpool,
            tc.tile_pool(name="po", bufs=4, space=bass.MemorySpace.PSUM) as popool,
        ):
            # PE p-state warmup: harmless matmuls on a memset tile keep the
            # tensor engine busy through the initial DMA wait so real work
            # starts at full clock.
            if WARMUP:
                wsrc = cpool.tile([F_, 256], f16, tag="wsrc")
                nc.gpsimd.memset(wsrc[:], 0.0)
                wps = popool.tile([128, IH // 128, VA_W], f32, tag="po",
                                  name="wps")
                for i in range(WARMUP):
                    nc.tensor.matmul(wps[:, 0, :], wsrc[:, 0:128],
                                     wsrc[:, 0:VA_W], start=True,
                                     stop=True)

            # --- input DMAs, ordered for earliest compute start ---
            kt = cpool.tile([F_, H * N], f16, tag="kt")
            qt = cpool.tile([F_, H * IH], f16, tag="qt")
            at_t = []
            for e in range(E):
                at = cpool.tile([128, JB * IH], f16, tag=f"at{e}")
                at_t.append(at)
            Q = 2 * IH                                           # quarter
            nc.sync.dma_start(kt[:, 0:N], kt_d[:, 0:N])          # h0 first
            nc.sync.dma_start(qt[:, 0:IH], qt_d[:, 0:IH])
            dg = cpool.tile([128, 128], f16, tag="dg")
            nc.sync.dma_start(dg[:], dg_d[:])
            for q in range(2):                                   # quarters 0,1
                for e in range(2):
                    nc.sync.dma_start(at_t[e][:, q * Q:(q + 1) * Q],
                                      at_d[e, :, q * Q:(q + 1) * Q])
            nc.sync.dma_start(kt[:, N:2 * N], kt_d[:, N:2 * N])  # h1
            nc.sync.dma_start(qt[:, IH:2 * IH], qt_d[:, IH:2 * IH])
            for q in range(2, 4):                                # quarters 2,3
                for e in range(2):
                    nc.sync.dma_start(at_t[e][:, q * Q:(q + 1) * Q],
                                      at_d[e, :, q * Q:(q + 1) * Q])
            nc.sync.dma_start(kt[:, 2 * N:], kt_d[:, 2 * N:])    # h2, h3
            nc.sync.dma_start(qt[:, 2 * IH:], qt_d[:, 2 * IH:])
            nc.sync.dma_start(at_t[2][:, 0:4 * IH], at_d[2, :, 0:4 * IH])
            nc.sync.dma_start(at_t[2][:, 4 * IH:], at_d[2, :, 4 * IH:])
            nc.sync.dma_start(at_t[3][:], at_d[3])
            va = cpool.tile([128, EH * JB * VA_W], f16, tag="va")
            nc.sync.dma_start(va[:], va_d[:])
            mbias = cpool.tile([128, 1], f32, tag="mbias")
            nc.gpsimd.memset(mbias[:], float(-MASK_BIG * scale))

            # --- phase P: pt[h][j-part, jb*IH + i] = exp(k_j . q_i * scale)
            pt_t = []
            for h in range(H):
                pt = ptpool.tile([128, JB * IH], f16, tag=f"pt{h}",
                                 name=f"pt{h}")
                pt_t.append(pt)

            def qk_pair(h, jbp, st, start=True, stop=True):
                for half in range(2):
                    jb = 2 * jbp + half
                    nc.tensor.matmul(
                        st[:, half * IH:(half + 1) * IH],
                        kt[:, h * N + jb * 128:h * N + (jb + 1) * 128],
                        qt[:, h * IH:(h + 1) * IH],
                        start=start, stop=stop)

            # --- phase C ---
            half = EH * F_ // 2
            stage = cpool.tile([128, IH // 128, EH * F_], f16, tag="stage")

            nb = IH // 128
            po_map = {}

            def po_group(h, e, tt):
                vab = (e * H + h) * JB * VA_W
                po = popool.tile([128, nb, VA_W], f32, tag="po")
                for ib in range(nb):
                    for jb in range(JB):
                        nc.tensor.matmul(
                            po[:, ib, :],
                            tt[:, jb * IH + ib * 128:jb * IH + ib * 128 + 128],
                            va[:, vab + jb * VA_W:vab + (jb + 1) * VA_W],
                            start=(jb == 0), stop=(jb == JB - 1))
                po_map[h, e] = po

            Relu = mybir.ActivationFunctionType.Relu

            def div_group(h, e, on_act=False):
                # divide+ReLU via per-partition scalar pointer on the DVE
                # (scalar operands are exempt from the one-PSUM-input rule);
                # late groups run as recip+relu on the Act engine, which has
                # drained its exp queue by then.
                col = (e * H + h) * F_
                po = po_map[h, e]
                rec = cpool.tile([128, nb, 1], f32, tag="rec", bufs=16)
                nc.vector.reciprocal(rec[:], po[:, :, F_:F_ + 1])
                for ib in range(nb):
                    if on_act:
                        nc.scalar.activation(
                            stage[:, ib, col:col + F_], po[:, ib, 0:F_],
                            Relu, scale=rec[:, ib, :])
                    else:
                        nc.vector.tensor_scalar(
                            stage[:, ib, col:col + F_], po[:, ib, 0:F_],
                            rec[:, ib, :], 0.0, mybir.AluOpType.mult,
                            mybir.AluOpType.max)

            tt_map = {}

            def fused_chunk(h, e, jbp):
                if (h, e) not in tt_map:
                    tt_map[h, e] = ttpool.tile([128, JB * IH], f16, tag="tt",
                                               name=f"ttf{h}{e}")
                tt = tt_map[h, e]
                st = stpool.tile([128, 2 * IH], f32, tag="st", name="stf")
                for half in range(2):
                    jb = 2 * jbp + half
                    sl = st[:, half * IH:(half + 1) * IH]
                    nc.tensor.matmul(
                        sl, kt[:, h * N + jb * 128:h * N + (jb + 1) * 128],
                        qt[:, h * IH:(h + 1) * IH], start=True, stop=False)
                    nc.tensor.matmul(
                        sl, dg[:], at_t[e][:, jb * IH:(jb + 1) * IH],
                        start=False, stop=True)
                nc.scalar.activation(
                    tt[:, jbp * 2 * IH:(jbp + 1) * 2 * IH], st[:], Exp,
                    scale=scale, bias=mbias[:])

            # phase P exps
            for h in range(H):
                for jbp in range(JB // 2):
                    st = stpool.tile([128, 2 * IH], f32, tag="st", name="stm")
                    qk_pair(h, jbp, st)
                    nc.scalar.activation(
                        pt_t[h][:, jbp * 2 * IH:(jbp + 1) * 2 * IH], st[:],
                        Exp, scale=scale)

            # DVE stream: chunked (h<2, e<2) ramp tracking the exp pipeline,
            # then whole-tile multiplies in readiness order.
            for h in range(2):
                for e in range(2):
                    tt_map[h, e] = ttpool.tile([128, JB * IH], f16, tag="tt",
                                               name=f"tt{h}{e}")
            for h in range(2):
                for jbp in range(JB // 2):
                    for e in range(2):
                        c0 = jbp * 2 * IH
                        nc.vector.tensor_mul(
                            tt_map[h, e][:, c0:c0 + 2 * IH],
                            at_t[e][:, c0:c0 + 2 * IH],
                            pt_t[h][:, c0:c0 + 2 * IH])

            # PE/Act/Pool stream: remaining fused chunks and po groups in
            # predicted-runnability order; output DMAs for each column half
            # right after that half's last contributing divide.
            f1, f2, f3 = FUSED
            emission = EMISSION or [
                ("po", 0, 0), ("po", 0, 1), ("po", 1, 0), ("po", 1, 1),
                ("f", *f2, 0), ("mul", 0, 2), ("po", 0, 2),
                ("f", *f2, 1), ("mul", 1, 2), ("po", 1, 2),
                ("f", *f2, 2), ("mul", 0, 3), ("po", 0, 3),
                ("f", *f2, 3), ("mul", 3, 0), ("po", 3, 0),
                ("div", 0, 0), ("div", 0, 1),
                ("mul", *f1), ("po", *f1),
                ("div", 1, 0), ("div", 1, 1),
                ("div", 0, 2), ("div", 1, 2),
                ("f", *f3, 0), ("mul", 2, 0), ("po", 2, 0),
                ("div", 0, 3),
                ("f", *f3, 1), ("mul", 1, 3), ("po", 1, 3), ("div", 3, 0),
                ("f", *f3, 2), ("mul", 2, 1), ("po", 2, 1), ("div", 2, 0),
                ("div", *f1),
                ("f", *f3, 3), ("mul", 2, 2), ("po", 2, 2), ("div", 1, 3),
                ("div", 2, 1),
                ("po", *f2), ("div", *f2),
                ("mul", 2, 3), ("po", 2, 3),
                ("po", *f3), ("div", 2, 2), ("div", *f3), ("div", 2, 3),
                ("out", 0), ("out", 1),
            ]
            ndiv = 0
            for act in emission:
                if act[0] == "f":
                    fused_chunk(act[1], act[2], act[3])
                elif act[0] == "mul":
                    h, e = act[1], act[2]
                    tt_map[h, e] = ttpool.tile([128, JB * IH], f16, tag="tt",
                                               name=f"ttm{h}{e}")
                    nc.vector.tensor_mul(tt_map[h, e][:], at_t[e][:],
                                         pt_t[h][:])
                elif act[0] == "po":
                    po_group(act[1], act[2], tt_map[act[1], act[2]])
                elif act[0] == "div":
                    late = act in (("div", 2, 0), ("div", 3, 0),
                                   ("div", 2, 1), ("div", 1, 3),
                                   ("div", *f1))
                    with tc.tile_wait_until((20.0 + 1.6 * ndiv) / 1000.0):
                        div_group(act[1], act[2], on_act=late)
                    ndiv += 1
                elif act[0] == "out":
                    for ib in (act[1], act[1] + 2):
                        nc.sync.dma_start(out_d[ib], stage[:, ib, :])

    nc.compile()
    return nc


def _prep_core_inputs(b, ih, X, A, kernel_w, biases, aks, akn):
    i0 = ih * IH
    Xb = X[b]                                        # [N, F]
    # qt[kk, h*IH + i] = sum_f X[i0+i, f] aks[h, f, kk]
    qt = np.einsum("nf,hfk->khn", Xb[i0:i0 + IH], aks).reshape(F_, H * IH)
    kt = np.einsum("nf,hfk->khn", Xb, akn).reshape(F_, H * N)
    v = np.einsum("nf,hfk->hnk", Xb, kernel_w)       # [H, N, F_]
    va = np.empty((E, H, JB, 128, VA_W), np.float32)
    for e in range(E):
        for h in range(H):
            vb = v[h] + biases[e, h][None, :]        # [N, F_]
            va[e, h, :, :, :F_] = vb.reshape(JB, 128, F_)
            va[e, h, :, :, F_] = 1.0
    # va2[p, ((e*H+h)*JB+jb)*33 + c]
    va2 = va.transpose(3, 0, 1, 2, 4).reshape(128, EH * JB * VA_W)
    # at[e, p, jb*IH + i] = A[b, e, i0+i, jb*128+p]
    at = (A[b, :, i0:i0 + IH, :]                     # [E, i, j]
          .transpose(0, 2, 1)                        # [E, j, i]
          .reshape(E, JB, 128, IH)
          .transpose(0, 2, 1, 3)                     # [E, p, jb, i]
          .reshape(E, 128, JB * IH))
    dg = np.diag(np.full(128, MASK_BIG, np.float32))
    return {"qt": qt.astype(np.float16),
            "kt": kt.astype(np.float16),
            "va": va2.astype(np.float16),
            "at": at.astype(np.float16),
            "dg": dg.astype(np.float16)}


def kernel(X, A, kernel, biases, attn_kernel_self, attn_kernel_neighs,
           attn_biases):
    global _compiled
    from concourse import bass_utils

    if _compiled is None:
        _compiled = _build()

    in_maps = [
        _prep_core_inputs(c // 2, c % 2, np.asarray(X), np.asarray(A),
                          np.asarray(kernel), np.asarray(biases),
                          np.asarray(attn_kernel_self),
                          np.asarray(attn_kernel_neighs))
        for c in range(NCORES)
    ]
    res = bass_utils.run_bass_kernel_spmd(_compiled, in_maps,
                                          core_ids=list(range(NCORES)))
    out = np.empty((B, N, EH * F_), np.float32)
    for c in range(NCORES):
        b, ih = c // 2, c % 2
        out[b, ih * IH:(ih + 1) * IH, :] = np.asarray(
            res.results[c]["out"]).reshape(IH, EH * F_).astype(np.float32)
    return out
